# revision 21
# baseline (speedup 1.0000x reference)
"""Trainium2 Bass kernel for a GPT-2 style transformer block.

Problem: x[2,2048,1024], 16 heads, causal attention, GELU(tanh) MLP, f32.

Sharding (8 NeuronCores):
  - Tokens are data-parallel: core c owns batch c//4, token rows
    512*(c%4) .. 512*(c%4)+512.  QKV, W_o, and the MLP are computed on the
    core's own 512 tokens with full (replicated) weights.
  - Attention is head-parallel: core c keeps heads 2c, 2c+1 and computes full
    causal attention for them over all tokens; K^T, Q^T, V cross via three
    pipelined fp8 AllToAlls, unnormalized AV sums + softmax reciprocal
    row-sums return via two more (one per head half).
  - LayerNorms are FOLDED into the following matmul: the LN weight is folded
    into W on the host, the matmul runs on the raw (un-normalized) residual
    stream, a rank-1 (-colsum(W) x mu) matmul appended to each PSUM
    accumulation group handles mean subtraction, and a per-token rstd
    broadcast multiply finishes the job.  This removes the LN normalize
    stage from the critical path entirely - only the cheap stats chain
    (sum / sum-of-squares via ones-matmuls) remains.
  - K,Q projections run as fp8 DoubleRow matmuls (256-deep contraction per
    pass, 2x bf16 rate).  V projection and attention probs are fp8 too
    (exp is computed as exp(s/sqrt(Dh) - 2.5); the offset cancels exactly
    after the row-sum normalization and keeps probs under the fp8e4 max).
    The MLP stays bf16 (fp8 there blows the 2e-2 error budget).
  - Softmax reciprocals are computed on the SOURCE core's DVE
    (reciprocal_approx_fast) and shipped with the y halves, so the ScalarE
    runs Exp only during attention and GELU during the MLP - no table
    thrash.  ACT table sets are preloaded with dummy activations during
    natural idle windows.
  - No DMA descriptors are issued from the ScalarE queue (they cost ~0.6us
    each and were starving the exp stream); Sync/Vector/GpSimd carry them.
  - Weights are uploaded pre-arranged (host-transposed into the exact
    [chunk, partition, kblock, col] SBUF layouts) so every weight DMA is a
    single long-contiguous-line transfer.
"""

import math
from contextlib import ExitStack

import ml_dtypes
import numpy as np

import concourse.bass as bass
import concourse.tile as tile
from concourse import mybir
from concourse import bacc
from concourse.bass_utils import run_bass_kernel_spmd
from concourse.masks import make_identity

F32 = mybir.dt.float32
BF16 = mybir.dt.bfloat16
F8 = mybir.dt.float8e4
AF = mybir.ActivationFunctionType
ALU = mybir.AluOpType
DR = mybir.MatmulPerfMode.DoubleRow

B, T, C = 2, 2048, 1024
H, DH = 16, 64
NCORES = 8
TOK = 512              # tokens per core
NCH = C // 128         # 8 feature chunks of the residual stream
FC4 = 4 * C            # 4096
RG = [list(range(NCORES))]
EXP_OFF = 2.5          # exp(s - EXP_OFF): cancels after normalization,
                       # keeps probs < fp8e4 max (240)
ISQ = 1.0 / math.sqrt(DH)

_compiled = {}


def _build():
    nc = bacc.Bacc(
        "TRN2",
        target_bir_lowering=False,
        debug=False,
        enable_asserts=False,
        num_devices=NCORES,
    )

    io = {}

    def din(name, shape, dt):
        io[name] = nc.dram_tensor(name, shape, dt, kind="ExternalInput").ap()

    din("xT_bf", [C, TOK], BF16)
    din("x8p", [4, 128, 2, TOK], F8)
    din("Wkq", [4, 128, 2, 2 * C], F8)        # device cols: 0:C = K, C:2C = Q
    din("Wv", [4, 128, 2, C], BF16)
    din("Wo", [4, 128, 2, C], BF16)
    din("Wfc", [8, 128, 8, 512], BF16)
    din("Wpj", [2, 8, 128, 4, 512], BF16)
    din("ncs_kq", [1, 2 * C], BF16)           # -colsum(Wkq_f8), K then Q
    din("csv", [1, C], BF16)                  # +colsum(Wv_bf)
    din("ncs_f", [1, FC4], BF16)              # -colsum(Wfc_bf)
    din("b_kq", [2 * C], F32)                 # effective biases (ln_b folded)
    din("b_v", [1, C], F32)
    din("b_o", [C], F32)
    din("b_fc", [FC4], F32)
    din("b_proj", [C], F32)
    io["out_T"] = nc.dram_tensor("out_T", [C, TOK], F32, kind="ExternalOutput").ap()

    with tile.TileContext(nc) as tc:
        _body(tc, io)
    nc.compile()
    return nc


def _body(tc, io):
    nc = tc.nc
    out_T = io["out_T"]

    ctx = ExitStack()
    persist = ctx.enter_context(tc.tile_pool(name="persist", bufs=1))
    dram = ctx.enter_context(tc.tile_pool(name="dram", bufs=1, space="DRAM"))
    xT_pool = ctx.enter_context(tc.tile_pool(name="xT_pool", bufs=1))

    # ---- collective buffers ----
    contrib_d = dram.tile([8, 128], BF16, name="contrib_d")
    gath_d = dram.tile([8, 128], BF16, name="gath_d")
    contrib_k = dram.tile([C, TOK], F8, name="contrib_k")
    contrib_q = dram.tile([C, TOK], F8, name="contrib_q")
    contrib_v = dram.tile([8 * TOK, 128], F8, name="contrib_v")
    contrib_yA = dram.tile([8 * 65, TOK], BF16, name="contrib_yA")
    contrib_yB = dram.tile([8 * 65, TOK], BF16, name="contrib_yB")
    gath_k = dram.tile([C, TOK], F8, name="gath_k")
    gath_q = dram.tile([C, TOK], F8, name="gath_q")
    gath_v = dram.tile([8 * TOK, 128], F8, name="gath_v")
    gath_yA = dram.tile([8 * 65, TOK], BF16, name="gath_yA")
    gath_yB = dram.tile([8 * 65, TOK], BF16, name="gath_yB")

    # ---- constants ----
    ident_bf = persist.tile([128, 128], BF16, name="ident_bf")
    make_identity(nc, ident_bf)
    # tiny all-to-all issued immediately: starts the one-time collective entry
    # barrier (~40us) ASAP and warms the ncfw path so the first real exchange
    # runs at full bandwidth (measured 3x faster K a2a with this in place)
    nc.sync.dma_start(contrib_d, ident_bf[0:8, 0:128])
    nc.gpsimd.collective_compute(
        "AllToAll", ALU.bypass, replica_groups=RG,
        ins=[contrib_d.opt()], outs=[gath_d.opt()],
    )
    ones_col = persist.tile([128, 1], F32, name="ones_col")
    nc.vector.memset(ones_col, 1.0)
    ones_col_bf = persist.tile([128, 1], BF16, name="ones_col_bf")
    nc.vector.memset(ones_col_bf, 1.0)
    ones_row = persist.tile([1, 128], F32, name="ones_row")
    nc.vector.memset(ones_row, 1.0)
    ones_row_bf = persist.tile([1, 128], BF16, name="ones_row_bf")
    nc.vector.memset(ones_row_bf, 1.0)
    one_f32 = persist.tile([1, 1], F32, name="one_f32")
    nc.vector.memset(one_f32, 1.0)
    selA_bf = persist.tile([1, 128], BF16, name="selA_bf")
    nc.vector.memset(selA_bf[0:1, 0:64], 1.0)
    nc.vector.memset(selA_bf[0:1, 64:128], 0.0)
    selB_bf = persist.tile([1, 128], BF16, name="selB_bf")
    nc.vector.memset(selB_bf[0:1, 0:64], 0.0)
    nc.vector.memset(selB_bf[0:1, 64:128], 1.0)
    eps_t = persist.tile([1, 1], F32, name="eps_t")
    nc.vector.memset(eps_t, 1e-5)
    noff_t = persist.tile([128, 1], F32, name="noff_t")
    nc.vector.memset(noff_t, -EXP_OFF)

    # preload the square/sqrt table sets before the LN1 stats need them
    tw1 = persist.tile([1, 1], BF16, name="tw1")
    nc.scalar.activation(tw1, one_f32, AF.Square)
    tw2 = persist.tile([1, 1], BF16, name="tw2")
    nc.scalar.activation(tw2, one_f32, AF.Sqrt)

    # ---- small params (gpsimd software-DGE; keeps HW queues free) ----
    bkq_s = persist.tile([128, 16], F32, name="bkq_s")
    bo_s = persist.tile([128, NCH], F32, name="bo_s")
    bf_s = persist.tile([128, 32], F32, name="bf_s")
    bp_s = persist.tile([128, NCH], F32, name="bp_s")
    for t, src in (
        (bkq_s, io["b_kq"]),
        (bo_s, io["b_o"]),
        (bf_s, io["b_fc"]),
        (bp_s, io["b_proj"]),
    ):
        nc.gpsimd.dma_start(t, src.rearrange("(a b) -> b a", b=128))
    ncs_kq = persist.tile([1, 2 * C], BF16, name="ncs_kq")
    nc.gpsimd.dma_start(ncs_kq, io["ncs_kq"])
    csv = persist.tile([1, C], BF16, name="csv")
    nc.gpsimd.dma_start(csv, io["csv"])
    ncs_f = persist.tile([1, FC4], BF16, name="ncs_f")
    nc.gpsimd.dma_start(ncs_f, io["ncs_f"])
    bv_f = persist.tile([1, C], F32, name="bv_f")
    nc.gpsimd.dma_start(bv_f, io["b_v"])
    bv_bf = persist.tile([1, C], BF16, name="bv_bf")
    nc.vector.tensor_copy(bv_bf, bv_f)

    def a2a(cin, cout):
        nc.gpsimd.collective_compute(
            "AllToAll", ALU.bypass, replica_groups=RG,
            ins=[cin.opt()], outs=[cout.opt()],
        )

    # ---- x arrives pre-transposed from the host: bf16 (residual + V path)
    #      and fp8 pre-paired (K,Q DoubleRow path) ----
    xT = [xT_pool.tile([128, TOK], BF16, name=f"xT{c}") for c in range(NCH)]
    for c in range(NCH):
        eng = nc.sync if c % 2 == 0 else nc.scalar
        eng.dma_start(xT[c], io["xT_bf"][c * 128 : (c + 1) * 128, :])
    x8t = [xT_pool.tile([128, 2, TOK], F8, name=f"x8t{k}") for k in range(4)]
    for k in range(4):
        nc.sync.dma_start(x8t[k], io["x8p"][k])

    # ---- LN1 statistics (no normalize - folded into QKV) ----
    ln_ctx = ExitStack()
    sq_pool = ln_ctx.enter_context(tc.tile_pool(name="sq_pool", bufs=3))
    small = persist  # small stat tiles live in persist
    st_ps = ln_ctx.enter_context(tc.tile_pool(name="st_ps", bufs=2, space="PSUM"))
    bc_ps = ln_ctx.enter_context(tc.tile_pool(name="bc_ps", bufs=1, space="PSUM"))

    ps_s = st_ps.tile([1, TOK], F32, name="ps_s", tag="st")
    ps_q = st_ps.tile([1, TOK], F32, name="ps_q", tag="st")
    for c in range(NCH):
        sq_t = sq_pool.tile([128, TOK], BF16, name=f"sq{c}", tag="sq")
        nc.scalar.activation(sq_t, xT[c], AF.Square)
        nc.tensor.matmul(ps_s, ones_col_bf, xT[c],
                         start=(c == 0), stop=(c == NCH - 1))
        nc.tensor.matmul(ps_q, ones_col_bf, sq_t,
                         start=(c == 0), stop=(c == NCH - 1))

    mu_bf = persist.tile([1, TOK], BF16, name="mu_bf")
    nmu_bf = persist.tile([1, TOK], BF16, name="nmu_bf")
    mu_f = persist.tile([1, TOK], F32, name="mu_f")
    msq = persist.tile([1, TOK], F32, name="msq")
    var = persist.tile([1, TOK], F32, name="var")
    rstd = persist.tile([1, TOK], F32, name="rstd")
    nc.vector.tensor_scalar_mul(mu_f, ps_s, 1.0 / C)
    nc.vector.tensor_copy(mu_bf, mu_f)
    nc.vector.tensor_scalar_mul(nmu_bf, ps_s, -1.0 / C)
    nc.vector.tensor_scalar_mul(msq, ps_q, 1.0 / C)
    nc.vector.tensor_mul(var, mu_f, mu_f)
    nc.vector.tensor_sub(var, msq, var)
    nc.scalar.activation(rstd, var, AF.Sqrt, bias=eps_t)
    nc.vector.reciprocal_approx_fast(rstd, rstd)

    # per-token rstd broadcast [128, TOK] (f32, lives in SBUF)
    rstd_bc = persist.tile([128, TOK], F32, name="rstd_bc")
    ps_rb = bc_ps.tile([128, TOK], F32, name="ps_rb", tag="bc_big")
    nc.tensor.matmul(ps_rb, ones_row, rstd, start=True, stop=True)
    nc.vector.tensor_copy(rstd_bc, ps_rb)
    # rstd transposed to [128 tok, 4 blk] (per-partition ACT scale for V)
    rstdT = persist.tile([128, 4], F32, name="rstdT")
    for tb in range(4):
        ps_t = bc_ps.tile([128, 1], F32, name=f"ps_t{tb}", tag="bc_t")
        nc.tensor.matmul(ps_t, rstd[0:1, tb * 128 : (tb + 1) * 128], one_f32,
                         start=True, stop=True)
        nc.vector.tensor_copy(rstdT[:, tb : tb + 1], ps_t)
    # b_v broadcast [128 tok, C] bf16
    bv_bc = persist.tile([128, C], BF16, name="bv_bc")
    for hb in range(2):
        ps_bv = bc_ps.tile([128, TOK], F32, name=f"ps_bv{hb}", tag="bc_bv", bufs=2)
        nc.tensor.matmul(ps_bv, ones_row_bf,
                         bv_bf[0:1, hb * 512 : (hb + 1) * 512],
                         start=True, stop=True)
        nc.vector.tensor_copy(bv_bc[:, hb * 512 : (hb + 1) * 512], ps_bv)
    ln_ctx.close()

    # ---- QKV ----
    qkv_ctx = ExitStack()
    wkq_pool = qkv_ctx.enter_context(tc.tile_pool(name="wkq_pool", bufs=1))
    qkv_sb = qkv_ctx.enter_context(tc.tile_pool(name="qkv_sb", bufs=4))
    qkv_ps = qkv_ctx.enter_context(tc.tile_pool(name="qkv_ps", bufs=5, space="PSUM"))

    wkq = [wkq_pool.tile([128, 2, 2 * C], F8, name=f"wkq{k}") for k in range(4)]
    for k in range(4):
        eng = nc.sync if k % 2 == 0 else nc.scalar
        eng.dma_start(wkq[k], io["Wkq"][k])

    # K chunks (device cols 0:C) then Q chunks (C:2C), fp8 DoubleRow.
    for g in range(4):
        for jj in range(4):
            j = 4 * g + jj          # absolute 128-col chunk, 0..15
            ps = qkv_ps.tile([128, TOK], F32, name=f"ps_kq{j}", tag="ps_qkv")
            for kk in range(4):
                nc.tensor.matmul(
                    ps, wkq[kk][:, :, j * 128 : (j + 1) * 128], x8t[kk],
                    start=(kk == 0), stop=False, perf_mode=DR,
                )
            nc.tensor.matmul(ps, ncs_kq[0:1, j * 128 : (j + 1) * 128], mu_bf,
                             start=False, stop=True)
            tmp = qkv_sb.tile([128, TOK], BF16, name=f"kqt{j}", tag="kqt")
            nc.vector.tensor_mul(tmp, ps, rstd_bc)
            o_t = qkv_sb.tile([128, TOK], F8, name=f"kqo{j}", tag="kqo")
            nc.scalar.activation(o_t, tmp, AF.Identity, bias=bkq_s[:, j : j + 1])
            contrib, row = (contrib_k, 128 * j) if j < 8 else (contrib_q, 128 * (j - 8))
            eng = nc.scalar if jj % 2 == 0 else nc.sync
            eng.dma_start(contrib[row : row + 128, :], o_t)
        if g == 1:
            a2a(contrib_k, gath_k)
        if g == 3:
            a2a(contrib_q, gath_q)

    # V token-major (bf16 matmul, LN-folded) -> fp8 contribs
    wv_pool = qkv_ctx.enter_context(tc.tile_pool(name="wv_pool", bufs=1))
    wv = [wv_pool.tile([128, 2, C], BF16, name=f"wv{k}") for k in range(4)]
    for k in range(4):
        eng = nc.sync if k % 2 == 0 else nc.scalar
        eng.dma_start(wv[k], io["Wv"][k])
    for og in range(2):
        for t in range(4):
            ps_v = qkv_ps.tile([128, TOK], F32, name=f"ps_v{og}_{t}", tag="ps_qkv")
            for c in range(NCH):
                nc.tensor.matmul(
                    ps_v, xT[c][:, t * 128 : (t + 1) * 128],
                    wv[c // 2][:, c % 2, og * 512 : (og + 1) * 512],
                    start=(c == 0), stop=False,
                )
            nc.tensor.matmul(ps_v, nmu_bf[0:1, t * 128 : (t + 1) * 128],
                             csv[0:1, og * 512 : (og + 1) * 512],
                             start=False, stop=True)
            tmpv = qkv_sb.tile([128, TOK], BF16, name=f"vt{og}_{t}", tag="kqt")
            nc.scalar.activation(tmpv, ps_v, AF.Identity,
                                 scale=rstdT[:, t : t + 1])
            v_o = qkv_sb.tile([128, TOK], F8, name=f"vo{og}_{t}", tag="kqo")
            nc.vector.tensor_add(v_o, tmpv, bv_bc[:, og * 512 : (og + 1) * 512])
            for hp in range(4):
                base = (4 * og + hp) * TOK + t * 128
                nc.sync.dma_start(
                    contrib_v[base : base + 128, :],
                    v_o[:, hp * 128 : (hp + 1) * 128],
                )
    a2a(contrib_v, gath_v)
    # preload the exp table set while waiting on the K/Q exchange
    expwarm = qkv_sb.tile([1, 1], BF16, name="expwarm", tag="actwarm")
    nc.scalar.activation(expwarm, one_f32, AF.Exp)
    qkv_ctx.close()

    # ---- head-parallel causal attention (heads 2c, 2c+1) ----
    att_ctx = ExitStack()
    att_k = att_ctx.enter_context(tc.tile_pool(name="att_k", bufs=2))
    att_v = att_ctx.enter_context(tc.tile_pool(name="att_v", bufs=2))
    att_t = att_ctx.enter_context(tc.tile_pool(name="att_t", bufs=4))
    att_sp = att_ctx.enter_context(tc.tile_pool(name="att_sp", bufs=2, space="PSUM"))
    att_av = att_ctx.enter_context(tc.tile_pool(name="att_av", bufs=2, space="PSUM"))

    k_sbs, v_sbs, q_ts = {}, {}, {}
    for b in range(B):
        k_sb = []
        for i in range(4):
            r = 4 * b + i
            kt_t = att_k.tile([128, 512], F8,
                              name=f"k_sb{b}_{i}", tag=f"k_sb{i}")
            eng = nc.sync if i % 2 == 0 else nc.scalar
            eng.dma_start(kt_t, gath_k[r * 128 : r * 128 + 128, :])
            k_sb.append(kt_t)
        k_sbs[b] = k_sb
        qts = []
        for qb in range(4):
            qT_t = att_t.tile([128, 512], F8, name=f"qT_t{b}_{qb}",
                              tag="qT_t", bufs=8)
            eng = nc.sync if qb % 2 == 0 else nc.scalar
            eng.dma_start(
                qT_t, gath_q[(4 * b + qb) * 128 : (4 * b + qb) * 128 + 128, :]
            )
            qts.append(qT_t)
        q_ts[b] = qts

    for b in range(B):
        v_sb = []
        for kt in range(16):
            r = 4 * b + kt // 4
            vt = att_v.tile([128, 130], F8, name=f"v_sb{b}_{kt}",
                            tag=f"v_sb{kt}")
            vv = vt.rearrange("p (a d) -> p a d", a=2)
            nc.vector.memset(vv[:, :, 64:65], 1.0)
            vbase = r * TOK + (kt % 4) * 128
            nc.sync.dma_start(
                vv[:, :, 0:64],
                gath_v[vbase : vbase + 128, :].rearrange("p (a d) -> p a d", a=2),
            )
            v_sb.append(vt)
        v_sbs[b] = v_sb

    # units: S tiles packed into <=3 psum banks (each matmul output must stay
    # within one 512-col bank); one Exp call per unit.
    pend = []
    unit_id = [0]
    avkey = {}

    def issue_av(u):
        b, qb, a, tiles, pT2, nkt = u
        avp = avkey[(b, qb, a)]
        for (kt, lo, off, w) in tiles:
            nc.tensor.matmul(
                avp[:, lo:], v_sbs[b][kt][:, 65 * a : 65 * a + 65],
                pT2[:, off : off + w],
                start=(kt == 0), stop=(kt == nkt - 1),
            )
        last_kt = tiles[-1][0]
        if last_kt == nkt - 1:
            avkey.pop((b, qb, a))
            y_sb = att_t.tile([65, TOK], BF16, name=f"y{b}_{qb}_{a}", tag="y_sb")
            nc.vector.tensor_copy(y_sb, avp)
            j = 4 * b + qb
            contrib = contrib_yA if a == 0 else contrib_yB
            nc.sync.dma_start(contrib[65 * j : 65 * j + 65, :], y_sb)

    for a in range(2):
        lookahead = 11 if a == 0 else 3
        for b in range(B):
            k_sb = k_sbs[b]
            qts = q_ts[b]
            for qb in range(4):
                nkt = 4 * qb + 4
                avkey[(b, qb, a)] = att_av.tile(
                    [65, TOK], F32, name=f"avp{b}_{qb}_{a}", tag="avp"
                )
                tl = []
                for kt in range(nkt):
                    r = kt - 4 * qb
                    lo = 128 * r if r > 0 else 0
                    tl.append((kt, r, lo, 512 - lo))
                # pack tiles into units of <=3 psum banks; within a bank,
                # tiles pack while their widths sum <=512
                units = []
                cur, nbanks, bank_off, bank_used = [], 0, 0, 512
                for (kt, r, lo, w) in tl:
                    if bank_used + w <= 512 and cur:
                        off = bank_off + bank_used
                        bank_used += w
                    else:
                        if nbanks == 3:
                            units.append(cur)
                            cur, nbanks = [], 0
                        off = nbanks * 512
                        bank_off = off
                        bank_used = w
                        nbanks += 1
                    cur.append((kt, r, lo, off, w))
                if cur:
                    units.append(cur)

                for ut in units:
                    uw = max(off + w for (_, _, _, off, w) in ut)
                    sp2 = att_sp.tile([128, 1536], F32,
                                      name=f"sp{unit_id[0]}", tag="sp")
                    pT2 = att_t.tile([128, 1536], F8,
                                     name=f"pT{unit_id[0]}", tag="pT", bufs=16)
                    unit_id[0] += 1
                    for (kt, r, lo, off, w) in ut:
                        nc.tensor.matmul(
                            sp2[:, off : off + w],
                            k_sb[kt // 4][64 * a : 64 * a + 64,
                                          (kt % 4) * 128 : (kt % 4) * 128 + 128],
                            qts[qb][64 * a : 64 * a + 64, lo:],
                            start=True, stop=True,
                        )
                    nc.scalar.activation(
                        pT2[:, 0:uw], sp2[:, 0:uw], AF.Exp,
                        scale=ISQ, bias=noff_t,
                    )
                    for (kt, r, lo, off, w) in ut:
                        if r >= 0:
                            nc.gpsimd.affine_select(
                                out=pT2[:, off : off + w],
                                in_=pT2[:, off : off + w],
                                compare_op=ALU.is_ge, fill=0.0,
                                base=-(128 * r - lo), channel_multiplier=-1,
                                pattern=[[1, w]],
                            )
                    pend.append((b, qb, a,
                                 [(kt, lo, off, w) for (kt, r, lo, off, w) in ut],
                                 pT2, nkt))
                    if len(pend) > lookahead:
                        issue_av(pend.pop(0))
        while pend:
            issue_av(pend.pop(0))
        a2a(contrib_yA if a == 0 else contrib_yB,
            gath_yA if a == 0 else gath_yB)
    att_ctx.close()

    # ---- y arrives with reciprocals; normalize, W_o + residual, LN2 stats ----
    mm_ctx = ExitStack()
    x2T_pool = mm_ctx.enter_context(tc.tile_pool(name="x2T_pool", bufs=1))
    mm_sb = mm_ctx.enter_context(tc.tile_pool(name="mm_sb", bufs=3))
    mm_ps = mm_ctx.enter_context(tc.tile_pool(name="mm_ps", bufs=4, space="PSUM"))
    x2T = [x2T_pool.tile([128, TOK], F32, name=f"x2T{c}") for c in range(NCH)]
    x2b = [x2T_pool.tile([128, TOK], BF16, name=f"x2b{c}") for c in range(NCH)]
    ln2_sb = mm_ctx.enter_context(tc.tile_pool(name="ln2_sb", bufs=3))

    wo_pool = mm_ctx.enter_context(tc.tile_pool(name="wo_pool", bufs=1))
    wo = [wo_pool.tile([128, 2, C], BF16, name=f"wo{k}") for k in range(4)]
    for k in range(4):
        eng = nc.sync if k % 2 == 0 else nc.scalar
        eng.dma_start(wo[k], io["Wo"][k])

    with (
        tc.tile_pool(name="yT_pool", bufs=1) as yT_pool,
        tc.tile_pool(name="rb_ps", bufs=2, space="PSUM") as rb_ps,
        tc.tile_pool(name="ln2_ps", bufs=2, space="PSUM") as ln2_ps,
    ):
        yT = [yT_pool.tile([128, TOK], BF16, name=f"yT{r}") for r in range(NCH)]
        rsA = [yT_pool.tile([1, TOK], BF16, name=f"rsA{r}") for r in range(NCH)]
        rsB = [yT_pool.tile([1, TOK], BF16, name=f"rsB{r}") for r in range(NCH)]
        rrA = [yT_pool.tile([1, TOK], BF16, name=f"rrA{r}") for r in range(NCH)]
        rrB = [yT_pool.tile([1, TOK], BF16, name=f"rrB{r}") for r in range(NCH)]
        rf = [yT_pool.tile([1, TOK], F32, name=f"rf{r}") for r in range(NCH)]
        for r in range(NCH):
            nc.sync.dma_start(yT[r][0:64, :], gath_yA[65 * r : 65 * r + 64, :])
            nc.scalar.dma_start(yT[r][64:128, :], gath_yB[65 * r : 65 * r + 64, :])
            nc.sync.dma_start(rsA[r], gath_yA[65 * r + 64 : 65 * r + 65, :])
            nc.scalar.dma_start(rsB[r], gath_yB[65 * r + 64 : 65 * r + 65, :])
        # preload the sqrt table set (ACT idle here; exp is done)
        sqwarm = mm_sb.tile([1, 1], BF16, name="sqwarm", tag="actwarm2")
        nc.scalar.activation(sqwarm, one_f32, AF.Sqrt)
        for r in range(NCH):
            # receiver-side softmax reciprocals (DVE; partition-0 tiles)
            nc.vector.tensor_copy(rf[r], rsA[r])
            nc.vector.reciprocal_approx_fast(rf[r], rf[r])
            nc.vector.tensor_copy(rrA[r], rf[r])
            nc.vector.tensor_copy(rf[r], rsB[r])
            nc.vector.reciprocal_approx_fast(rf[r], rf[r])
            nc.vector.tensor_copy(rrB[r], rf[r])
            ps_rb2 = rb_ps.tile([128, TOK], F32, name=f"ps_yrb{r}", tag="yrb")
            nc.tensor.matmul(ps_rb2, selA_bf, rrA[r], start=True, stop=False)
            nc.tensor.matmul(ps_rb2, selB_bf, rrB[r], start=False, stop=True)
            nc.vector.tensor_mul(yT[r], yT[r], ps_rb2)

        ps_s2 = ln2_ps.tile([1, TOK], F32, name="ps_s2", tag="ln2_ps")
        ps_q2 = ln2_ps.tile([1, TOK], F32, name="ps_q2", tag="ln2_ps")
        for og in range(2):
            for jj in range(4):
                oc = 4 * og + jj
                ps_o = mm_ps.tile([128, TOK], F32, name=f"ps_o{oc}", tag="ps_mm")
                for k in range(NCH):
                    nc.tensor.matmul(
                        ps_o, wo[k // 2][:, k % 2, oc * 128 : (oc + 1) * 128],
                        yT[k],
                        start=(k == 0), stop=(k == NCH - 1),
                    )
                nc.vector.scalar_tensor_tensor(
                    x2T[oc], ps_o, bo_s[:, oc : oc + 1], xT[oc],
                    op0=ALU.add, op1=ALU.add,
                )
                nc.vector.tensor_copy(x2b[oc], x2T[oc])
                sq2 = ln2_sb.tile([128, TOK], BF16, name=f"sq2{oc}", tag="ln2sq")
                nc.vector.tensor_mul(sq2, x2b[oc], x2b[oc])
                nc.tensor.matmul(ps_s2, ones_col_bf, x2b[oc],
                                 start=(oc == 0), stop=(oc == NCH - 1))
                nc.tensor.matmul(ps_q2, ones_col_bf, sq2,
                                 start=(oc == 0), stop=(oc == NCH - 1))

        mu2_bf = persist.tile([1, TOK], BF16, name="mu2_bf")
        mu2_f = persist.tile([1, TOK], F32, name="mu2_f")
        msq2 = persist.tile([1, TOK], F32, name="msq2")
        var2 = persist.tile([1, TOK], F32, name="var2")
        rstd2 = persist.tile([1, TOK], F32, name="rstd2")
        nc.vector.tensor_scalar_mul(mu2_f, ps_s2, 1.0 / C)
        nc.vector.tensor_copy(mu2_bf, mu2_f)
        nc.vector.tensor_scalar_mul(msq2, ps_q2, 1.0 / C)
        nc.vector.tensor_mul(var2, mu2_f, mu2_f)
        nc.vector.tensor_sub(var2, msq2, var2)
        nc.scalar.activation(rstd2, var2, AF.Sqrt, bias=eps_t)
        nc.vector.reciprocal_approx_fast(rstd2, rstd2)
        rstd2_bc = persist.tile([128, TOK], F32, name="rstd2_bc")
        ps_rb3 = rb_ps.tile([128, TOK], F32, name="ps_rb3", tag="yrb")
        nc.tensor.matmul(ps_rb3, ones_row, rstd2, start=True, stop=True)
        nc.vector.tensor_copy(rstd2_bc, ps_rb3)
        # preload the gelu table set before the first FC output lands
        gwarm = mm_sb.tile([1, 1], BF16, name="gwarm", tag="actwarm3")
        nc.scalar.activation(gwarm, one_f32, AF.Gelu_apprx_tanh)

    # ---- FC + GELU (LN2 folded) ----
    fc_ctx = ExitStack()
    fc_pool = fc_ctx.enter_context(tc.tile_pool(name="fc_pool", bufs=32))
    wf_pool = fc_ctx.enter_context(tc.tile_pool(name="wf_pool", bufs=3))
    fcT = []
    for fg in range(NCH):
        wf = wf_pool.tile([128, 8, 512], BF16, name=f"wf{fg}", tag="wf")
        eng = nc.sync if fg % 2 == 0 else nc.scalar
        eng.dma_start(wf, io["Wfc"][fg])
        for jj in range(4):
            fcol = 4 * fg + jj
            ps_f = mm_ps.tile([128, TOK], F32, name=f"ps_f{fcol}", tag="ps_mm")
            for k in range(NCH):
                nc.tensor.matmul(
                    ps_f, wf[:, k, jj * 128 : (jj + 1) * 128], x2b[k],
                    start=(k == 0), stop=False,
                )
            nc.tensor.matmul(ps_f, ncs_f[0:1, fcol * 128 : (fcol + 1) * 128],
                             mu2_bf, start=False, stop=True)
            tmpf = ln2_sb.tile([128, TOK], BF16, name=f"tf{fcol}", tag="ln2sq")
            nc.vector.tensor_mul(tmpf, ps_f, rstd2_bc)
            fc_t = fc_pool.tile([128, TOK], BF16, name=f"fcT{fcol}", tag="fcT")
            nc.scalar.activation(
                fc_t, tmpf, AF.Gelu_apprx_tanh, bias=bf_s[:, fcol : fcol + 1]
            )
            fcT.append(fc_t)

    # ---- proj + residual ----
    wp_pool = fc_ctx.enter_context(tc.tile_pool(name="wp_pool", bufs=4))
    for og in range(2):
        ps_p = [
            mm_ps.tile([128, TOK], F32, name=f"ps_p{og}_{jj}", tag="ps_mm")
            for jj in range(4)
        ]
        for fkk in range(8):
            wp = wp_pool.tile([128, 4, 512], BF16, name=f"wp{og}_{fkk}", tag="wp")
            eng = nc.sync if fkk % 2 == 0 else nc.scalar
            eng.dma_start(wp, io["Wpj"][og, fkk])
            for jj in range(4):
                for j in range(4):
                    fk = 4 * fkk + j
                    nc.tensor.matmul(
                        ps_p[jj], wp[:, j, jj * 128 : (jj + 1) * 128],
                        fcT[fk],
                        start=(fk == 0), stop=(fk == 31),
                    )
        for jj in range(4):
            oc = 4 * og + jj
            o_sb = mm_sb.tile([128, TOK], F32, name=f"o_sb{oc}", tag="o_sb")
            nc.vector.scalar_tensor_tensor(
                o_sb, ps_p[jj], bp_s[:, oc : oc + 1], x2T[oc],
                op0=ALU.add, op1=ALU.add,
            )
            nc.sync.dma_start(out_T[oc * 128 : (oc + 1) * 128, 0 : TOK // 2],
                              o_sb[:, 0 : TOK // 2])
            nc.scalar.dma_start(out_T[oc * 128 : (oc + 1) * 128, TOK // 2 : TOK],
                                o_sb[:, TOK // 2 : TOK])

    fc_ctx.close()
    mm_ctx.close()
    ctx.close()


def _get_nc():
    if "nc" not in _compiled:
        _compiled["nc"] = _build()
    return _compiled["nc"]


F8NP = ml_dtypes.float8_e4m3
BFNP = ml_dtypes.bfloat16


def _prep_shared(inputs):
    f32 = np.float32
    W_attn = np.asarray(inputs["W_attn"], f32)
    ln1_w = np.asarray(inputs["ln1_w"], f32)
    ln1_b = np.asarray(inputs["ln1_b"], f32)
    b_attn = np.asarray(inputs["b_attn"], f32)
    W_o = np.asarray(inputs["W_o"], f32)
    ln2_w = np.asarray(inputs["ln2_w"], f32)
    ln2_b = np.asarray(inputs["ln2_b"], f32)
    W_fc = np.asarray(inputs["W_fc"], f32)
    W_proj = np.asarray(inputs["W_proj"], f32)

    Wa = W_attn * ln1_w[:, None]
    b_eff = b_attn + ln1_b @ W_attn
    # device col order: K (orig 1024:2048) then Q (orig 0:1024)
    Wkq8 = np.concatenate([Wa[:, C : 2 * C], Wa[:, 0:C]], axis=1).astype(F8NP)
    ncs_kq = -(Wkq8.astype(f32).sum(0))
    b_kq = np.concatenate([b_eff[C : 2 * C], b_eff[0:C]])
    Wv_bf = Wa[:, 2 * C :].astype(BFNP)
    csv = Wv_bf.astype(f32).sum(0)
    b_v = b_eff[2 * C :]

    Wf_bf = (W_fc * ln2_w[:, None]).astype(BFNP)
    ncs_f = -(Wf_bf.astype(f32).sum(0))
    b_fc_eff = np.asarray(inputs["b_fc"], f32) + ln2_b @ W_fc

    shared = {
        "Wkq": np.ascontiguousarray(
            Wkq8.reshape(4, 2, 128, 2 * C).transpose(0, 2, 1, 3)),
        "Wv": np.ascontiguousarray(
            Wv_bf.reshape(4, 2, 128, C).transpose(0, 2, 1, 3)),
        "Wo": np.ascontiguousarray(
            W_o.astype(BFNP).reshape(4, 2, 128, C).transpose(0, 2, 1, 3)),
        "Wfc": np.ascontiguousarray(
            Wf_bf.reshape(8, 128, 8, 512).transpose(2, 1, 0, 3)),
        "Wpj": np.ascontiguousarray(
            W_proj.astype(BFNP).reshape(8, 4, 128, 2, 512)
            .transpose(3, 0, 2, 1, 4)),
        "ncs_kq": np.ascontiguousarray(ncs_kq.astype(BFNP).reshape(1, -1)),
        "csv": np.ascontiguousarray(csv.astype(BFNP).reshape(1, -1)),
        "ncs_f": np.ascontiguousarray(ncs_f.astype(BFNP).reshape(1, -1)),
        "b_kq": np.ascontiguousarray(b_kq),
        "b_v": np.ascontiguousarray(b_v.reshape(1, -1)),
        "b_o": np.ascontiguousarray(np.asarray(inputs["b_o"], f32)),
        "b_fc": np.ascontiguousarray(b_fc_eff),
        "b_proj": np.ascontiguousarray(np.asarray(inputs["b_proj"], f32)),
    }
    return shared


def kernel(**inputs):
    nc = _get_nc()
    x = np.ascontiguousarray(np.asarray(inputs["x"], dtype=np.float32))
    shared = _prep_shared(inputs)
    in_maps = []
    for c in range(NCORES):
        b, qb = c // 4, c % 4
        m = dict(shared)
        xT = np.ascontiguousarray(
            x[b, 512 * qb : 512 * (qb + 1), :].T.astype(BFNP))
        m["xT_bf"] = xT
        x8 = xT.astype(F8NP)
        m["x8p"] = np.ascontiguousarray(
            x8.reshape(4, 2, 128, TOK).transpose(0, 2, 1, 3))
        in_maps.append(m)
    res = run_bass_kernel_spmd(nc, in_maps, core_ids=list(range(NCORES)))
    _compiled["last_results"] = res
    out = np.empty((B, T, C), dtype=np.float32)
    for c, r in enumerate(res.results):
        b, qb = c // 4, c % 4
        out[b, 512 * qb : 512 * (qb + 1), :] = r["out_T"].T
    return out


# revision 25
# speedup vs baseline: 1.0562x; 1.0562x over previous
"""Trainium2 Bass kernel for a GPT-2 style transformer block.

Problem: x[2,2048,1024], 16 heads, causal attention, GELU(tanh) MLP, f32.

Sharding (8 NeuronCores):
  - Tokens are data-parallel: core c owns batch c//4, token rows
    512*(c%4) .. 512*(c%4)+512.  QKV, W_o, and the MLP are computed on the
    core's own 512 tokens with full (replicated) weights.
  - Attention is head-parallel: core c keeps heads 2c, 2c+1 and computes full
    causal attention for them over all tokens; K^T, Q^T, V cross via three
    pipelined fp8 AllToAlls, unnormalized AV sums + softmax reciprocal
    row-sums return via two more (one per head half).
  - LayerNorms are FOLDED into the following matmul: the LN weight is folded
    into W on the host, the matmul runs on the raw (un-normalized) residual
    stream, a rank-1 (-colsum(W) x mu) matmul appended to each PSUM
    accumulation group handles mean subtraction, and a per-token rstd
    broadcast multiply finishes the job.  This removes the LN normalize
    stage from the critical path entirely - only the cheap stats chain
    (sum / sum-of-squares via ones-matmuls) remains.
  - K,Q projections run as fp8 DoubleRow matmuls (256-deep contraction per
    pass, 2x bf16 rate).  V projection and attention probs are fp8 too
    (exp is computed as exp(s/sqrt(Dh) - 2.5); the offset cancels exactly
    after the row-sum normalization and keeps probs under the fp8e4 max).
    The MLP stays bf16 (fp8 there blows the 2e-2 error budget).
  - Softmax reciprocals are computed on the SOURCE core's DVE
    (reciprocal_approx_fast) and shipped with the y halves, so the ScalarE
    runs Exp only during attention and GELU during the MLP - no table
    thrash.  ACT table sets are preloaded with dummy activations during
    natural idle windows.
  - No DMA descriptors are issued from the ScalarE queue (they cost ~0.6us
    each and were starving the exp stream); Sync/Vector/GpSimd carry them.
  - Weights are uploaded pre-arranged (host-transposed into the exact
    [chunk, partition, kblock, col] SBUF layouts) so every weight DMA is a
    single long-contiguous-line transfer.
"""

import math
from contextlib import ExitStack

import ml_dtypes
import numpy as np

import concourse.bass as bass
import concourse.tile as tile
from concourse import mybir
from concourse import bacc
from concourse.bass_utils import run_bass_kernel_spmd
from concourse.masks import make_identity

F32 = mybir.dt.float32
BF16 = mybir.dt.bfloat16
F8 = mybir.dt.float8e4
AF = mybir.ActivationFunctionType
ALU = mybir.AluOpType
DR = mybir.MatmulPerfMode.DoubleRow

B, T, C = 2, 2048, 1024
H, DH = 16, 64
NCORES = 8
TOK = 512              # tokens per core
NCH = C // 128         # 8 feature chunks of the residual stream
FC4 = 4 * C            # 4096
RG = [list(range(NCORES))]
EXP_OFF = 2.5          # exp(s - EXP_OFF): cancels after normalization,
                       # keeps probs < fp8e4 max (240)
ISQ = 1.0 / math.sqrt(DH)

_compiled = {}


def _build():
    nc = bacc.Bacc(
        "TRN2",
        target_bir_lowering=False,
        debug=False,
        enable_asserts=False,
        num_devices=NCORES,
    )

    io = {}

    def din(name, shape, dt):
        io[name] = nc.dram_tensor(name, shape, dt, kind="ExternalInput").ap()

    din("xT_bf", [C, TOK], BF16)
    din("x8p", [4, 128, 2, TOK], F8)
    din("Wkq", [4, 128, 2, 2 * C], F8)        # device cols: 0:C = K, C:2C = Q
    din("Wv", [4, 128, 2, C], BF16)
    din("Wo", [4, 128, 2, C], BF16)
    din("Wfc", [8, 128, 8, 512], BF16)
    din("Wpj", [2, 8, 128, 4, 512], BF16)
    din("ncs_kq", [1, 2 * C], BF16)           # -colsum(Wkq_f8), K then Q
    din("csv", [1, C], BF16)                  # +colsum(Wv_bf)
    din("ncs_f", [1, FC4], BF16)              # -colsum(Wfc_bf)
    din("b_kq", [2 * C], F32)                 # effective biases (ln_b folded)
    din("b_v", [1, C], F32)
    din("b_o", [C], F32)
    din("b_fc", [FC4], F32)
    din("b_proj", [C], F32)
    io["out_T"] = nc.dram_tensor("out_T", [C, TOK], F32, kind="ExternalOutput").ap()

    with tile.TileContext(nc) as tc:
        _body(tc, io)
    nc.compile()
    return nc


def _body(tc, io):
    nc = tc.nc
    out_T = io["out_T"]

    ctx = ExitStack()
    persist = ctx.enter_context(tc.tile_pool(name="persist", bufs=1))
    dram = ctx.enter_context(tc.tile_pool(name="dram", bufs=1, space="DRAM"))
    xT_pool = ctx.enter_context(tc.tile_pool(name="xT_pool", bufs=1))

    # ---- collective buffers ----
    contrib_d = dram.tile([8, 128], BF16, name="contrib_d")
    gath_d = dram.tile([8, 128], BF16, name="gath_d")
    contrib_k = dram.tile([C, TOK], F8, name="contrib_k")
    contrib_q = dram.tile([C, TOK], F8, name="contrib_q")
    contrib_v = dram.tile([8 * TOK, 128], F8, name="contrib_v")
    contrib_yA = dram.tile([8 * 65, TOK], BF16, name="contrib_yA")
    contrib_yB = dram.tile([8 * 65, TOK], BF16, name="contrib_yB")
    gath_k = dram.tile([C, TOK], F8, name="gath_k")
    gath_q = dram.tile([C, TOK], F8, name="gath_q")
    gath_v = dram.tile([8 * TOK, 128], F8, name="gath_v")
    gath_yA = dram.tile([8 * 65, TOK], BF16, name="gath_yA")
    gath_yB = dram.tile([8 * 65, TOK], BF16, name="gath_yB")

    # ---- constants ----
    # tiny all-to-all issued first: starts the one-time collective entry
    # barrier (~40us) ASAP and warms the ncfw path so the first real exchange
    # runs at full bandwidth (measured 3x faster K a2a with this in place)
    dseed = persist.tile([8, 128], BF16, name="dseed")
    nc.vector.memset(dseed, 0.0)
    nc.sync.dma_start(contrib_d, dseed)
    nc.gpsimd.collective_compute(
        "AllToAll", ALU.bypass, replica_groups=RG,
        ins=[contrib_d.opt()], outs=[gath_d.opt()],
    )
    ident_bf = persist.tile([128, 128], BF16, name="ident_bf")
    make_identity(nc, ident_bf)
    ones_col = persist.tile([128, 1], F32, name="ones_col")
    nc.vector.memset(ones_col, 1.0)
    ones_col_bf = persist.tile([128, 1], BF16, name="ones_col_bf")
    nc.vector.memset(ones_col_bf, 1.0)
    ones_row = persist.tile([1, 128], F32, name="ones_row")
    nc.vector.memset(ones_row, 1.0)
    ones_row_bf = persist.tile([1, 128], BF16, name="ones_row_bf")
    nc.vector.memset(ones_row_bf, 1.0)
    one_f32 = persist.tile([1, 1], F32, name="one_f32")
    nc.vector.memset(one_f32, 1.0)
    selA_bf = persist.tile([1, 128], BF16, name="selA_bf")
    nc.vector.memset(selA_bf[0:1, 0:64], 1.0)
    nc.vector.memset(selA_bf[0:1, 64:128], 0.0)
    selB_bf = persist.tile([1, 128], BF16, name="selB_bf")
    nc.vector.memset(selB_bf[0:1, 0:64], 0.0)
    nc.vector.memset(selB_bf[0:1, 64:128], 1.0)
    eps_t = persist.tile([1, 1], F32, name="eps_t")
    nc.vector.memset(eps_t, 1e-5)
    noff_t = persist.tile([128, 1], F32, name="noff_t")
    nc.vector.memset(noff_t, -EXP_OFF)

    # preload the square/sqrt table sets before the LN1 stats need them
    tw1 = persist.tile([1, 1], BF16, name="tw1")
    nc.scalar.activation(tw1, one_f32, AF.Square)
    tw2 = persist.tile([1, 1], BF16, name="tw2")
    nc.scalar.activation(tw2, one_f32, AF.Sqrt)

    # ---- small params (gpsimd software-DGE; keeps HW queues free) ----
    bkq_s = persist.tile([128, 16], F32, name="bkq_s")
    bo_s = persist.tile([128, NCH], F32, name="bo_s")
    bf_s = persist.tile([128, 32], F32, name="bf_s")
    bp_s = persist.tile([128, NCH], F32, name="bp_s")
    for t, src in (
        (bkq_s, io["b_kq"]),
        (bo_s, io["b_o"]),
        (bf_s, io["b_fc"]),
        (bp_s, io["b_proj"]),
    ):
        nc.gpsimd.dma_start(t, src.rearrange("(a b) -> b a", b=128))
    ncs_kq = persist.tile([1, 2 * C], BF16, name="ncs_kq")
    nc.gpsimd.dma_start(ncs_kq, io["ncs_kq"])
    csv = persist.tile([1, C], BF16, name="csv")
    nc.gpsimd.dma_start(csv, io["csv"])
    ncs_f = persist.tile([1, FC4], BF16, name="ncs_f")
    nc.gpsimd.dma_start(ncs_f, io["ncs_f"])
    bv_f = persist.tile([1, C], F32, name="bv_f")
    nc.gpsimd.dma_start(bv_f, io["b_v"])
    bv_bf = persist.tile([1, C], BF16, name="bv_bf")
    nc.vector.tensor_copy(bv_bf, bv_f)

    def a2a(cin, cout):
        nc.gpsimd.collective_compute(
            "AllToAll", ALU.bypass, replica_groups=RG,
            ins=[cin.opt()], outs=[cout.opt()],
        )

    # ---- x arrives pre-transposed from the host: bf16 (residual + V path)
    #      and fp8 pre-paired (K,Q DoubleRow path) ----
    xT = [xT_pool.tile([128, TOK], BF16, name=f"xT{c}") for c in range(NCH)]
    for c in range(NCH):
        eng = nc.sync if c % 2 == 0 else nc.scalar
        eng.dma_start(xT[c], io["xT_bf"][c * 128 : (c + 1) * 128, :])
    x8t = [xT_pool.tile([128, 2, TOK], F8, name=f"x8t{k}") for k in range(4)]
    for k in range(4):
        nc.sync.dma_start(x8t[k], io["x8p"][k])

    # ---- LN1 statistics (no normalize - folded into QKV) ----
    ln_ctx = ExitStack()
    sq_pool = ln_ctx.enter_context(tc.tile_pool(name="sq_pool", bufs=3))
    small = persist  # small stat tiles live in persist
    st_ps = ln_ctx.enter_context(tc.tile_pool(name="st_ps", bufs=2, space="PSUM"))
    bc_ps = ln_ctx.enter_context(tc.tile_pool(name="bc_ps", bufs=1, space="PSUM"))

    ps_s = st_ps.tile([1, TOK], F32, name="ps_s", tag="st")
    ps_q = st_ps.tile([1, TOK], F32, name="ps_q", tag="st")
    for c in range(NCH):
        sq_t = sq_pool.tile([128, TOK], BF16, name=f"sq{c}", tag="sq")
        nc.scalar.activation(sq_t, xT[c], AF.Square)
        nc.tensor.matmul(ps_s, ones_col_bf, xT[c],
                         start=(c == 0), stop=(c == NCH - 1))
        nc.tensor.matmul(ps_q, ones_col_bf, sq_t,
                         start=(c == 0), stop=(c == NCH - 1))

    mu_bf = persist.tile([1, TOK], BF16, name="mu_bf")
    nmu_bf = persist.tile([1, TOK], BF16, name="nmu_bf")
    mu_f = persist.tile([1, TOK], F32, name="mu_f")
    msq = persist.tile([1, TOK], F32, name="msq")
    var = persist.tile([1, TOK], F32, name="var")
    rstd = persist.tile([1, TOK], F32, name="rstd")
    nc.vector.tensor_scalar_mul(mu_f, ps_s, 1.0 / C)
    nc.vector.tensor_copy(mu_bf, mu_f)
    nc.vector.tensor_scalar_mul(nmu_bf, ps_s, -1.0 / C)
    nc.vector.tensor_scalar_mul(msq, ps_q, 1.0 / C)
    nc.vector.tensor_mul(var, mu_f, mu_f)
    nc.vector.tensor_sub(var, msq, var)
    nc.scalar.activation(rstd, var, AF.Sqrt, bias=eps_t)
    nc.vector.reciprocal_approx_fast(rstd, rstd)

    # per-token rstd broadcast [128, TOK] (f32, lives in SBUF)
    rstd_bc = persist.tile([128, TOK], F32, name="rstd_bc")
    ps_rb = bc_ps.tile([128, TOK], F32, name="ps_rb", tag="bc_big")
    nc.tensor.matmul(ps_rb, ones_row, rstd, start=True, stop=True)
    nc.vector.tensor_copy(rstd_bc, ps_rb)
    # rstd transposed to [128 tok, 4 blk] (per-partition ACT scale for V)
    rstdT = persist.tile([128, 4], F32, name="rstdT")
    for tb in range(4):
        ps_t = bc_ps.tile([128, 1], F32, name=f"ps_t{tb}", tag="bc_t")
        nc.tensor.matmul(ps_t, rstd[0:1, tb * 128 : (tb + 1) * 128], one_f32,
                         start=True, stop=True)
        nc.vector.tensor_copy(rstdT[:, tb : tb + 1], ps_t)
    # b_v broadcast [128 tok, C] bf16
    bv_bc = persist.tile([128, C], BF16, name="bv_bc")
    for hb in range(2):
        ps_bv = bc_ps.tile([128, TOK], F32, name=f"ps_bv{hb}", tag="bc_bv", bufs=2)
        nc.tensor.matmul(ps_bv, ones_row_bf,
                         bv_bf[0:1, hb * 512 : (hb + 1) * 512],
                         start=True, stop=True)
        nc.vector.tensor_copy(bv_bc[:, hb * 512 : (hb + 1) * 512], ps_bv)
    ln_ctx.close()

    # ---- QKV ----
    qkv_ctx = ExitStack()
    wkq_pool = qkv_ctx.enter_context(tc.tile_pool(name="wkq_pool", bufs=1))
    qkv_sb = qkv_ctx.enter_context(tc.tile_pool(name="qkv_sb", bufs=4))
    qkv_ps = qkv_ctx.enter_context(tc.tile_pool(name="qkv_ps", bufs=5, space="PSUM"))

    wkq = [wkq_pool.tile([128, 2, 2 * C], F8, name=f"wkq{k}") for k in range(4)]
    for k in range(4):
        eng = nc.sync if k % 2 == 0 else nc.scalar
        eng.dma_start(wkq[k], io["Wkq"][k])

    # K chunks (device cols 0:C) then Q chunks (C:2C), fp8 DoubleRow.
    for g in range(4):
        for jj in range(4):
            j = 4 * g + jj          # absolute 128-col chunk, 0..15
            ps = qkv_ps.tile([128, TOK], F32, name=f"ps_kq{j}", tag="ps_qkv")
            for kk in range(4):
                nc.tensor.matmul(
                    ps, wkq[kk][:, :, j * 128 : (j + 1) * 128], x8t[kk],
                    start=(kk == 0), stop=False, perf_mode=DR,
                )
            nc.tensor.matmul(ps, ncs_kq[0:1, j * 128 : (j + 1) * 128], mu_bf,
                             start=False, stop=True)
            tmp = qkv_sb.tile([128, TOK], BF16, name=f"kqt{j}", tag="kqt")
            nc.vector.tensor_mul(tmp, ps, rstd_bc)
            o_t = qkv_sb.tile([128, TOK], F8, name=f"kqo{j}", tag="kqo")
            nc.scalar.activation(o_t, tmp, AF.Identity, bias=bkq_s[:, j : j + 1])
            contrib, row = (contrib_k, 128 * j) if j < 8 else (contrib_q, 128 * (j - 8))
            eng = nc.scalar if jj % 2 == 0 else nc.sync
            eng.dma_start(contrib[row : row + 128, :], o_t)
        if g == 1:
            a2a(contrib_k, gath_k)
        if g == 3:
            a2a(contrib_q, gath_q)

    # V token-major (bf16 matmul, LN-folded) -> fp8 contribs
    wv_pool = qkv_ctx.enter_context(tc.tile_pool(name="wv_pool", bufs=1))
    wv = [wv_pool.tile([128, 2, C], BF16, name=f"wv{k}") for k in range(4)]
    for k in range(4):
        eng = nc.sync if k % 2 == 0 else nc.scalar
        eng.dma_start(wv[k], io["Wv"][k])
    for og in range(2):
        for t in range(4):
            ps_v = qkv_ps.tile([128, TOK], F32, name=f"ps_v{og}_{t}", tag="ps_qkv")
            for c in range(NCH):
                nc.tensor.matmul(
                    ps_v, xT[c][:, t * 128 : (t + 1) * 128],
                    wv[c // 2][:, c % 2, og * 512 : (og + 1) * 512],
                    start=(c == 0), stop=False,
                )
            nc.tensor.matmul(ps_v, nmu_bf[0:1, t * 128 : (t + 1) * 128],
                             csv[0:1, og * 512 : (og + 1) * 512],
                             start=False, stop=True)
            tmpv = qkv_sb.tile([128, TOK], BF16, name=f"vt{og}_{t}", tag="kqt")
            nc.scalar.activation(tmpv, ps_v, AF.Identity,
                                 scale=rstdT[:, t : t + 1])
            v_o = qkv_sb.tile([128, TOK], F8, name=f"vo{og}_{t}", tag="kqo")
            nc.vector.tensor_add(v_o, tmpv, bv_bc[:, og * 512 : (og + 1) * 512])
            for hp in range(4):
                base = (4 * og + hp) * TOK + t * 128
                nc.sync.dma_start(
                    contrib_v[base : base + 128, :],
                    v_o[:, hp * 128 : (hp + 1) * 128],
                )
    a2a(contrib_v, gath_v)
    # preload the exp table set while waiting on the K/Q exchange
    expwarm = qkv_sb.tile([1, 1], BF16, name="expwarm", tag="actwarm")
    nc.scalar.activation(expwarm, one_f32, AF.Exp)
    qkv_ctx.close()

    # pools created before the attention pools so stack order holds
    wo_pool = ctx.enter_context(tc.tile_pool(name="wo_pool", bufs=1))
    wf_pool = ctx.enter_context(tc.tile_pool(name="wf_pool", bufs=3))

    # ---- head-parallel causal attention (heads 2c, 2c+1) ----
    att_ctx = ExitStack()
    att_k = att_ctx.enter_context(tc.tile_pool(name="att_k", bufs=2))
    att_v = att_ctx.enter_context(tc.tile_pool(name="att_v", bufs=2))
    att_t = att_ctx.enter_context(tc.tile_pool(name="att_t", bufs=4))
    att_sp = att_ctx.enter_context(tc.tile_pool(name="att_sp", bufs=2, space="PSUM"))
    att_av = att_ctx.enter_context(tc.tile_pool(name="att_av", bufs=2, space="PSUM"))

    k_sbs, v_sbs, q_ts = {}, {}, {}
    for b in range(B):
        k_sb = []
        for i in range(4):
            r = 4 * b + i
            kt_t = att_k.tile([128, 512], F8,
                              name=f"k_sb{b}_{i}", tag=f"k_sb{i}")
            eng = nc.sync if i % 2 == 0 else nc.scalar
            eng.dma_start(kt_t, gath_k[r * 128 : r * 128 + 128, :])
            k_sb.append(kt_t)
        k_sbs[b] = k_sb
        qts = []
        for qb in range(4):
            qT_t = att_t.tile([128, 512], F8, name=f"qT_t{b}_{qb}",
                              tag="qT_t", bufs=8)
            eng = nc.sync if qb % 2 == 0 else nc.scalar
            eng.dma_start(
                qT_t, gath_q[(4 * b + qb) * 128 : (4 * b + qb) * 128 + 128, :]
            )
            qts.append(qT_t)
        q_ts[b] = qts

    for b in range(B):
        v_sb = []
        for k2 in range(8):
            # inner dim padded to 128: the dual-fp8 LDWEIGHTS path requires
            # aligned k-pair strides (s3_lw_dual_fp8_restrictions)
            vp = att_v.tile([128, 2, 2, 128], F8, name=f"v_sb{b}_{k2}",
                            tag=f"v_sb{k2}")
            nc.vector.memset(vp[:, :, :, 64:65], 1.0)
            for j in range(2):
                kt = 2 * k2 + j
                r = 4 * b + kt // 4
                vbase = r * TOK + (kt % 4) * 128
                nc.sync.dma_start(
                    vp[:, j, :, 0:64],
                    gath_v[vbase : vbase + 128, :]
                    .rearrange("p (a d) -> p a d", a=2),
                )
            v_sb.append(vp)
        v_sbs[b] = v_sb

    # prefetch W_o and the first W_fc tiles now: the attention window is
    # DMA-light, and streaming them later collides with the y all-to-alls
    wo = [wo_pool.tile([128, 2, C], BF16, name=f"wo{k}") for k in range(4)]
    for k in range(4):
        nc.sync.dma_start(wo[k], io["Wo"][k])
    wf_pre = {}
    for fg in range(2):
        wf = wf_pool.tile([128, 8, 512], BF16, name=f"wf{fg}", tag="wf")
        nc.sync.dma_start(wf, io["Wfc"][fg])
        wf_pre[fg] = wf

    # units: S tiles packed into <=3 psum banks (each matmul output must stay
    # within one 512-col bank); one Exp call per unit.
    pend = []
    unit_id = [0]
    avkey = {}

    def issue_av(u):
        b, qb, a, tiles, pT2, nkt = u
        avp = avkey[(b, qb, a)]
        i = 0
        while i < len(tiles):
            kt, lo, off, w = tiles[i]
            if (i + 1 < len(tiles) and kt % 2 == 0 and w == 512
                    and tiles[i + 1][0] == kt + 1 and tiles[i + 1][3] == 512
                    and tiles[i + 1][2] == off + 512):
                # fp8 DoubleRow over the (kt, kt+1) pair
                nc.tensor.matmul(
                    avp, v_sbs[b][kt // 2][:, :, a, 0:65],
                    pT2[:, off : off + 1024].rearrange(
                        "p (j w) -> p j w", j=2),
                    start=(kt == 0), stop=(kt + 1 == nkt - 1),
                    perf_mode=DR,
                )
                i += 2
                continue
            nc.tensor.matmul(
                avp[:, lo:], v_sbs[b][kt // 2][:, kt % 2, a, 0:65],
                pT2[:, off : off + w],
                start=(kt == 0), stop=(kt == nkt - 1),
            )
            i += 1
        last_kt = tiles[-1][0]
        if last_kt == nkt - 1:
            avkey.pop((b, qb, a))
            y_sb = att_t.tile([65, TOK], BF16, name=f"y{b}_{qb}_{a}", tag="y_sb")
            nc.vector.tensor_copy(y_sb, avp)
            j = 4 * b + qb
            contrib = contrib_yA if a == 0 else contrib_yB
            nc.sync.dma_start(contrib[65 * j : 65 * j + 65, :], y_sb)

    for a in range(2):
        lookahead = 11 if a == 0 else 3
        for b in range(B):
            k_sb = k_sbs[b]
            qts = q_ts[b]
            for qb in range(4):
                nkt = 4 * qb + 4
                avkey[(b, qb, a)] = att_av.tile(
                    [65, TOK], F32, name=f"avp{b}_{qb}_{a}", tag="avp"
                )
                tl = []
                for kt in range(nkt):
                    r = kt - 4 * qb
                    lo = 128 * r if r > 0 else 0
                    tl.append((kt, r, lo, 512 - lo))
                # pack tiles into units of <=3 psum banks; within a bank,
                # tiles pack while their widths sum <=512
                units = []
                cur, nbanks, bank_off, bank_used = [], 0, 0, 512
                for (kt, r, lo, w) in tl:
                    if bank_used + w <= 512 and cur:
                        off = bank_off + bank_used
                        bank_used += w
                    else:
                        if nbanks == 3:
                            units.append(cur)
                            cur, nbanks = [], 0
                        off = nbanks * 512
                        bank_off = off
                        bank_used = w
                        nbanks += 1
                    cur.append((kt, r, lo, off, w))
                if cur:
                    units.append(cur)

                for ut in units:
                    uw = max(off + w for (_, _, _, off, w) in ut)
                    sp2 = att_sp.tile([128, 1536], F32,
                                      name=f"sp{unit_id[0]}", tag="sp")
                    pT2 = att_t.tile([128, 1536], F8,
                                     name=f"pT{unit_id[0]}", tag="pT", bufs=16)
                    unit_id[0] += 1
                    for (kt, r, lo, off, w) in ut:
                        nc.tensor.matmul(
                            sp2[:, off : off + w],
                            k_sb[kt // 4][64 * a : 64 * a + 64,
                                          (kt % 4) * 128 : (kt % 4) * 128 + 128],
                            qts[qb][64 * a : 64 * a + 64, lo:],
                            start=True, stop=True,
                        )
                    nc.scalar.activation(
                        pT2[:, 0:uw], sp2[:, 0:uw], AF.Exp,
                        scale=ISQ, bias=noff_t,
                    )
                    for (kt, r, lo, off, w) in ut:
                        if r >= 0:
                            nc.gpsimd.affine_select(
                                out=pT2[:, off : off + w],
                                in_=pT2[:, off : off + w],
                                compare_op=ALU.is_ge, fill=0.0,
                                base=-(128 * r - lo), channel_multiplier=-1,
                                pattern=[[1, w]],
                            )
                    pend.append((b, qb, a,
                                 [(kt, lo, off, w) for (kt, r, lo, off, w) in ut],
                                 pT2, nkt))
                    if len(pend) > lookahead:
                        issue_av(pend.pop(0))
        while pend:
            issue_av(pend.pop(0))
        a2a(contrib_yA if a == 0 else contrib_yB,
            gath_yA if a == 0 else gath_yB)
    att_ctx.close()

    # ---- y arrives with reciprocals; normalize, W_o + residual, LN2 stats ----
    mm_ctx = ExitStack()
    x2T_pool = mm_ctx.enter_context(tc.tile_pool(name="x2T_pool", bufs=1))
    mm_sb = mm_ctx.enter_context(tc.tile_pool(name="mm_sb", bufs=3))
    mm_ps = mm_ctx.enter_context(tc.tile_pool(name="mm_ps", bufs=4, space="PSUM"))
    x2T = [x2T_pool.tile([128, TOK], F32, name=f"x2T{c}") for c in range(NCH)]
    x2b = [x2T_pool.tile([128, TOK], BF16, name=f"x2b{c}") for c in range(NCH)]
    ln2_sb = mm_ctx.enter_context(tc.tile_pool(name="ln2_sb", bufs=3))

    with (
        tc.tile_pool(name="yT_pool", bufs=1) as yT_pool,
        tc.tile_pool(name="rb_ps", bufs=2, space="PSUM") as rb_ps,
        tc.tile_pool(name="ln2_ps", bufs=2, space="PSUM") as ln2_ps,
    ):
        yT = [yT_pool.tile([128, TOK], BF16, name=f"yT{r}") for r in range(NCH)]
        rsA = [yT_pool.tile([1, TOK], BF16, name=f"rsA{r}") for r in range(NCH)]
        rsB = [yT_pool.tile([1, TOK], BF16, name=f"rsB{r}") for r in range(NCH)]
        rrA = [yT_pool.tile([1, TOK], BF16, name=f"rrA{r}") for r in range(NCH)]
        rrB = [yT_pool.tile([1, TOK], BF16, name=f"rrB{r}") for r in range(NCH)]
        rf = [yT_pool.tile([1, TOK], F32, name=f"rf{r}") for r in range(NCH)]
        for r in range(NCH):
            nc.sync.dma_start(yT[r][0:64, :], gath_yA[65 * r : 65 * r + 64, :])
            nc.scalar.dma_start(yT[r][64:128, :], gath_yB[65 * r : 65 * r + 64, :])
            nc.sync.dma_start(rsA[r], gath_yA[65 * r + 64 : 65 * r + 65, :])
            nc.scalar.dma_start(rsB[r], gath_yB[65 * r + 64 : 65 * r + 65, :])
        # preload the sqrt table set (ACT idle here; exp is done)
        sqwarm = mm_sb.tile([1, 1], BF16, name="sqwarm", tag="actwarm2")
        nc.scalar.activation(sqwarm, one_f32, AF.Sqrt)
        for r in range(NCH):
            # receiver-side softmax reciprocals (DVE; partition-0 tiles)
            nc.vector.tensor_copy(rf[r], rsA[r])
            nc.vector.reciprocal_approx_fast(rf[r], rf[r])
            nc.vector.tensor_copy(rrA[r], rf[r])
            nc.vector.tensor_copy(rf[r], rsB[r])
            nc.vector.reciprocal_approx_fast(rf[r], rf[r])
            nc.vector.tensor_copy(rrB[r], rf[r])
            ps_rb2 = rb_ps.tile([128, TOK], F32, name=f"ps_yrb{r}", tag="yrb")
            nc.tensor.matmul(ps_rb2, selA_bf, rrA[r], start=True, stop=False)
            nc.tensor.matmul(ps_rb2, selB_bf, rrB[r], start=False, stop=True)
            nc.vector.tensor_mul(yT[r], yT[r], ps_rb2)

        ps_s2 = ln2_ps.tile([1, TOK], F32, name="ps_s2", tag="ln2_ps")
        ps_q2 = ln2_ps.tile([1, TOK], F32, name="ps_q2", tag="ln2_ps")
        for og in range(2):
            for jj in range(4):
                oc = 4 * og + jj
                ps_o = mm_ps.tile([128, TOK], F32, name=f"ps_o{oc}", tag="ps_mm")
                for k in range(NCH):
                    nc.tensor.matmul(
                        ps_o, wo[k // 2][:, k % 2, oc * 128 : (oc + 1) * 128],
                        yT[k],
                        start=(k == 0), stop=(k == NCH - 1),
                    )
                nc.vector.scalar_tensor_tensor(
                    x2T[oc], ps_o, bo_s[:, oc : oc + 1], xT[oc],
                    op0=ALU.add, op1=ALU.add,
                )
                nc.vector.tensor_copy(x2b[oc], x2T[oc])
                sq2 = ln2_sb.tile([128, TOK], BF16, name=f"sq2{oc}", tag="ln2sq")
                nc.vector.tensor_mul(sq2, x2b[oc], x2b[oc])
                nc.tensor.matmul(ps_s2, ones_col_bf, x2b[oc],
                                 start=(oc == 0), stop=(oc == NCH - 1))
                nc.tensor.matmul(ps_q2, ones_col_bf, sq2,
                                 start=(oc == 0), stop=(oc == NCH - 1))

        mu2_bf = persist.tile([1, TOK], BF16, name="mu2_bf")
        mu2_f = persist.tile([1, TOK], F32, name="mu2_f")
        msq2 = persist.tile([1, TOK], F32, name="msq2")
        var2 = persist.tile([1, TOK], F32, name="var2")
        rstd2 = persist.tile([1, TOK], F32, name="rstd2")
        nc.vector.tensor_scalar_mul(mu2_f, ps_s2, 1.0 / C)
        nc.vector.tensor_copy(mu2_bf, mu2_f)
        nc.vector.tensor_scalar_mul(msq2, ps_q2, 1.0 / C)
        nc.vector.tensor_mul(var2, mu2_f, mu2_f)
        nc.vector.tensor_sub(var2, msq2, var2)
        nc.scalar.activation(rstd2, var2, AF.Sqrt, bias=eps_t)
        nc.vector.reciprocal_approx_fast(rstd2, rstd2)
        rstd2_bc = persist.tile([128, TOK], F32, name="rstd2_bc")
        ps_rb3 = rb_ps.tile([128, TOK], F32, name="ps_rb3", tag="yrb")
        nc.tensor.matmul(ps_rb3, ones_row, rstd2, start=True, stop=True)
        nc.vector.tensor_copy(rstd2_bc, ps_rb3)
        # preload the gelu table set before the first FC output lands
        gwarm = mm_sb.tile([1, 1], BF16, name="gwarm", tag="actwarm3")
        nc.scalar.activation(gwarm, one_f32, AF.Gelu_apprx_tanh)

    # ---- FC + GELU (LN2 folded) ----
    fc_ctx = ExitStack()
    fc_pool = fc_ctx.enter_context(tc.tile_pool(name="fc_pool", bufs=32))
    fcT = []
    for fg in range(NCH):
        if fg in wf_pre:
            wf = wf_pre[fg]
        else:
            wf = wf_pool.tile([128, 8, 512], BF16, name=f"wf{fg}", tag="wf")
            eng = nc.sync if fg % 2 == 0 else nc.scalar
            eng.dma_start(wf, io["Wfc"][fg])
        for jj in range(4):
            fcol = 4 * fg + jj
            ps_f = mm_ps.tile([128, TOK], F32, name=f"ps_f{fcol}", tag="ps_mm")
            for k in range(NCH):
                nc.tensor.matmul(
                    ps_f, wf[:, k, jj * 128 : (jj + 1) * 128], x2b[k],
                    start=(k == 0), stop=False,
                )
            nc.tensor.matmul(ps_f, ncs_f[0:1, fcol * 128 : (fcol + 1) * 128],
                             mu2_bf, start=False, stop=True)
            tmpf = ln2_sb.tile([128, TOK], BF16, name=f"tf{fcol}", tag="ln2sq")
            nc.vector.tensor_mul(tmpf, ps_f, rstd2_bc)
            fc_t = fc_pool.tile([128, TOK], BF16, name=f"fcT{fcol}", tag="fcT")
            nc.scalar.activation(
                fc_t, tmpf, AF.Gelu_apprx_tanh, bias=bf_s[:, fcol : fcol + 1]
            )
            fcT.append(fc_t)

    # ---- proj + residual ----
    wp_pool = fc_ctx.enter_context(tc.tile_pool(name="wp_pool", bufs=4))
    for og in range(2):
        ps_p = [
            mm_ps.tile([128, TOK], F32, name=f"ps_p{og}_{jj}", tag="ps_mm")
            for jj in range(4)
        ]
        for fkk in range(8):
            wp = wp_pool.tile([128, 4, 512], BF16, name=f"wp{og}_{fkk}", tag="wp")
            eng = nc.sync if fkk % 2 == 0 else nc.scalar
            eng.dma_start(wp, io["Wpj"][og, fkk])
            for jj in range(4):
                for j in range(4):
                    fk = 4 * fkk + j
                    nc.tensor.matmul(
                        ps_p[jj], wp[:, j, jj * 128 : (jj + 1) * 128],
                        fcT[fk],
                        start=(fk == 0), stop=(fk == 31),
                    )
        for jj in range(4):
            oc = 4 * og + jj
            o_sb = mm_sb.tile([128, TOK], F32, name=f"o_sb{oc}", tag="o_sb")
            nc.vector.scalar_tensor_tensor(
                o_sb, ps_p[jj], bp_s[:, oc : oc + 1], x2T[oc],
                op0=ALU.add, op1=ALU.add,
            )
            nc.sync.dma_start(out_T[oc * 128 : (oc + 1) * 128, 0 : TOK // 2],
                              o_sb[:, 0 : TOK // 2])
            nc.scalar.dma_start(out_T[oc * 128 : (oc + 1) * 128, TOK // 2 : TOK],
                                o_sb[:, TOK // 2 : TOK])

    fc_ctx.close()
    mm_ctx.close()
    ctx.close()


def _get_nc():
    if "nc" not in _compiled:
        _compiled["nc"] = _build()
    return _compiled["nc"]


F8NP = ml_dtypes.float8_e4m3
BFNP = ml_dtypes.bfloat16


def _prep_shared(inputs):
    f32 = np.float32
    W_attn = np.asarray(inputs["W_attn"], f32)
    ln1_w = np.asarray(inputs["ln1_w"], f32)
    ln1_b = np.asarray(inputs["ln1_b"], f32)
    b_attn = np.asarray(inputs["b_attn"], f32)
    W_o = np.asarray(inputs["W_o"], f32)
    ln2_w = np.asarray(inputs["ln2_w"], f32)
    ln2_b = np.asarray(inputs["ln2_b"], f32)
    W_fc = np.asarray(inputs["W_fc"], f32)
    W_proj = np.asarray(inputs["W_proj"], f32)

    Wa = W_attn * ln1_w[:, None]
    b_eff = b_attn + ln1_b @ W_attn
    # device col order: K (orig 1024:2048) then Q (orig 0:1024)
    Wkq8 = np.concatenate([Wa[:, C : 2 * C], Wa[:, 0:C]], axis=1).astype(F8NP)
    ncs_kq = -(Wkq8.astype(f32).sum(0))
    b_kq = np.concatenate([b_eff[C : 2 * C], b_eff[0:C]])
    Wv_bf = Wa[:, 2 * C :].astype(BFNP)
    csv = Wv_bf.astype(f32).sum(0)
    b_v = b_eff[2 * C :]

    Wf_bf = (W_fc * ln2_w[:, None]).astype(BFNP)
    ncs_f = -(Wf_bf.astype(f32).sum(0))
    b_fc_eff = np.asarray(inputs["b_fc"], f32) + ln2_b @ W_fc

    shared = {
        "Wkq": np.ascontiguousarray(
            Wkq8.reshape(4, 2, 128, 2 * C).transpose(0, 2, 1, 3)),
        "Wv": np.ascontiguousarray(
            Wv_bf.reshape(4, 2, 128, C).transpose(0, 2, 1, 3)),
        "Wo": np.ascontiguousarray(
            W_o.astype(BFNP).reshape(4, 2, 128, C).transpose(0, 2, 1, 3)),
        "Wfc": np.ascontiguousarray(
            Wf_bf.reshape(8, 128, 8, 512).transpose(2, 1, 0, 3)),
        "Wpj": np.ascontiguousarray(
            W_proj.astype(BFNP).reshape(8, 4, 128, 2, 512)
            .transpose(3, 0, 2, 1, 4)),
        "ncs_kq": np.ascontiguousarray(ncs_kq.astype(BFNP).reshape(1, -1)),
        "csv": np.ascontiguousarray(csv.astype(BFNP).reshape(1, -1)),
        "ncs_f": np.ascontiguousarray(ncs_f.astype(BFNP).reshape(1, -1)),
        "b_kq": np.ascontiguousarray(b_kq),
        "b_v": np.ascontiguousarray(b_v.reshape(1, -1)),
        "b_o": np.ascontiguousarray(np.asarray(inputs["b_o"], f32)),
        "b_fc": np.ascontiguousarray(b_fc_eff),
        "b_proj": np.ascontiguousarray(np.asarray(inputs["b_proj"], f32)),
    }
    return shared


def kernel(**inputs):
    nc = _get_nc()
    x = np.ascontiguousarray(np.asarray(inputs["x"], dtype=np.float32))
    shared = _prep_shared(inputs)
    in_maps = []
    for c in range(NCORES):
        b, qb = c // 4, c % 4
        m = dict(shared)
        xT = np.ascontiguousarray(
            x[b, 512 * qb : 512 * (qb + 1), :].T.astype(BFNP))
        m["xT_bf"] = xT
        x8 = xT.astype(F8NP)
        m["x8p"] = np.ascontiguousarray(
            x8.reshape(4, 2, 128, TOK).transpose(0, 2, 1, 3))
        in_maps.append(m)
    res = run_bass_kernel_spmd(nc, in_maps, core_ids=list(range(NCORES)))
    _compiled["last_results"] = res
    out = np.empty((B, T, C), dtype=np.float32)
    for c, r in enumerate(res.results):
        b, qb = c // 4, c % 4
        out[b, 512 * qb : 512 * (qb + 1), :] = r["out_T"].T
    return out
